# revision 15
# baseline (speedup 1.0000x reference)
"""Trainium2 Bass kernel for nn_Chord_Note_Conv (ragged embedding-bag + conv1d stack).

Design (per core, data-parallel over batch: B=16 -> 2 batch rows/core, P=4096 positions):
  * The ragged note-embedding mean, the chord embedding lookup, AND conv1 are fused
    into sparse-histogram matmuls: for each position build a 1024-bin count row
    C'[pos, v] (note bins 0..831 scaled by 1/cnt, chord bin 832+chord = 1).
    Scatter index/value tables (dedup + 1/cnt scaling, pure int16/fp16 index
    bookkeeping) are precomputed on the host alongside the input relayout.
  * C' rows are built with the GPSIMD local_scatter instruction (per-partition
    independent scatter), transposed to [bins, positions] with the HWDGE xbar
    DMA transpose into an even/odd split layout (cols 0:256 even, 272:528 odd).
  * conv1 uses Winograd F(2,3): per 512-pos s-block, transform histogram cols
    d0 = x[2p-1]-x[2p+1], d1 = x[2p]+x[2p+1], d2 = x[2p+1]-x[2p], d3 = x[2p]-x[2p+2]
    (packed DVE ops thanks to the even/odd layout), then u_i = G_i^T d_i
    (4 GEMM groups of N=256 instead of 3 taps of N=512: 2/3 the PE work),
    y[2p] = u0+u1+u2, y[2p+1] = u1-u2-u3 combined in-place in PSUM on DVE,
    relu+bias via strided Scalar activations writing x2 interleaved.
  * conv2 + fc are plain PE matmuls; fc bias via an appended ones-row.
  * Everything on-chip is fp16 (fp32 PSUM accumulation) -> ~1e-4 rel err.
"""

import os

os.environ.setdefault("MYCRO_LOCAL_CACHE", "1")

import numpy as np

try:
    import concourse.bass as bass  # noqa: F401
except ImportError:
    import sys

    sys.path.insert(0, "/opt/trn_rl_repo")
    import concourse.bass as bass

from concourse import bacc, mybir, tile
from concourse.bass_utils import run_bass_kernel_spmd

FP16 = mybir.dt.float16   # NB: bf16 produced wrong results on HW (and was no faster)
F32 = mybir.dt.float32
I16 = mybir.dt.int16
ALU = mybir.AluOpType
RELU = mybir.ActivationFunctionType.Relu

CHORD_SIZE, NOTE_SIZE = 150, 832
B, S, N = 16, 2048, 16
NCORES = 8
BLOC = B // NCORES          # batch rows per core
P = BLOC * S                # positions per core = 4096
NT = P // 128               # pos tiles per core = 32
NSB = P // 512              # s-blocks per core = 8
SB_PER_BATCH = S // 512     # 4
NBINS = 1024                # 832 note + 150 chord + pad
NQ = NBINS // 128           # 8 v-chunks
CW = 544                    # ct s-block buffer width (E 0:257, O-guard 271, O 272:528)
TB = (0, 128, 272, 400)     # transpose col base per tile (E0, E1, O0, O1)


def _build_program():
    nc = bacc.Bacc("TRN2", target_bir_lowering=False, debug=False,
                   enable_asserts=False, num_devices=NCORES)

    # ---- DRAM I/O (flat 2D) ----
    d_sidx = nc.dram_tensor("sidx", [128, NT * 18], I16, kind="ExternalInput")
    d_sval = nc.dram_tensor("sval", [128, NT * 18], FP16, kind="ExternalInput")
    d_g1t = nc.dram_tensor("g1t", [128, 4 * NQ * 2 * 128], FP16, kind="ExternalInput")
    d_w2t = nc.dram_tensor("w2t", [128, 3 * 2 * 64], FP16, kind="ExternalInput")
    d_fcwb = nc.dram_tensor("fcwb", [65, 152], FP16, kind="ExternalInput")
    d_b1 = nc.dram_tensor("b1t", [128, 2], F32, kind="ExternalInput")
    d_b2 = nc.dram_tensor("b2t", [64, 1], F32, kind="ExternalInput")
    d_ones = nc.dram_tensor("ones16", [1, P], FP16, kind="ExternalInput")
    d_out = nc.dram_tensor("out", [P, CHORD_SIZE], FP16, kind="ExternalOutput")

    # ---- persistent SBUF ----
    def sb(name, shape, dt):
        return nc.alloc_sbuf_tensor(name, list(shape), dt).ap()

    s_g1t = sb("s_g1t", [128, 4 * NQ * 2 * 128], FP16)
    s_w2t = sb("s_w2t", [128, 3 * 2 * 64], FP16)
    s_fcwb = sb("s_fcwb", [65, 152], FP16)
    s_b1 = sb("s_b1", [128, 2], F32)
    s_b2 = sb("s_b2", [64, 1], F32)
    s_x3 = sb("s_x3", [65, P], FP16)
    s_sidx = sb("s_sidx", [128, NT * 18], I16)
    s_sval = sb("s_sval", [128, NT * 18], FP16)
    # 8 ct s-block buffers [128, NQ, CW] + 8 x2 buffers [128, 2, CW]
    s_ct = [sb(f"s_ct{i}", [128, NQ * CW], FP16) for i in range(NSB)]
    s_x2 = [sb(f"s_x2{i}", [128, 2 * CW], FP16) for i in range(NSB)]

    sidx3 = s_sidx.rearrange("p (c j) -> p c j", j=18)
    sval3 = s_sval.rearrange("p (c j) -> p c j", j=18)
    g1t5 = s_g1t.rearrange("p (i q c o) -> p i q c o", i=4, q=NQ, c=2)
    w2t4 = s_w2t.rearrange("p (k q o) -> p k q o", k=3, q=2)
    ct3 = [t.rearrange("p (q w) -> p q w", w=CW) for t in s_ct]
    x23 = [t.rearrange("p (q w) -> p q w", w=CW) for t in s_x2]

    with tile.TileContext(nc) as tc, \
         nc.allow_low_precision(reason="fp16 data, fp32 accumulation"):
        v = nc.vector
        # ---- input loads ----
        # scatter tables on the sync queue (first in line); weights on the
        # scalar queue so transposes aren't stuck behind the 2MB g1t.
        nc.sync.dma_start(s_sidx, d_sidx.ap())
        nc.sync.dma_start(s_sval, d_sval.ap())
        nc.scalar.dma_start(s_g1t, d_g1t.ap())  # big one first: semaphores
        nc.scalar.dma_start(s_b1, d_b1.ap())    # alias with transpose waits
        nc.scalar.dma_start(s_b2, d_b2.ap())
        nc.scalar.dma_start(s_fcwb, d_fcwb.ap())
        nc.scalar.dma_start(s_w2t, d_w2t.ap())
        nc.scalar.dma_start(s_x3[64:65, :], d_ones.ap())  # fc ones row

        # ---- histogram + transpose (even/odd split layout) ----
        # sb0's O-tiles + sb1's first tile transpose on the scalar queue: the
        # first conv1 needs 5 transposes and each blocks its queue ~1.3us.
        def hist_sb(cpool, sbk, engs=None):
            for t in range(4):
                ti = 4 * sbk + t
                ct_ = cpool.tile([128, NBINS], FP16, tag="c")
                nc.gpsimd.local_scatter(
                    ct_[:], sval3[:, ti, :], sidx3[:, ti, :],
                    channels=128, num_elems=NBINS, num_idxs=18)
                eng = engs[t] if engs else nc.sync
                eng.dma_start_transpose(
                    ct3[sbk][:, :, TB[t]:TB[t] + 128], ct_[:])

        with tc.tile_pool(name="cp", bufs=6) as cpool, \
             tc.tile_pool(name="dp", bufs=3) as dpool:
            dbufs = [None] * NSB

            # ---- Winograd F(2,3) input transform ----
            def dtrans(sbk):
                c = ct3[sbk]
                if sbk % SB_PER_BATCH == 0:
                    v.memset(c[:, :, 271:272], 0.0)          # O[-1] = 0
                else:
                    nc.scalar.copy(c[:, :, 271:272], ct3[sbk - 1][:, :, 527:528])
                if sbk % SB_PER_BATCH == SB_PER_BATCH - 1:
                    v.memset(c[:, :, 256:257], 0.0)          # E[256] = 0
                else:
                    nc.scalar.copy(c[:, :, 256:257], ct3[sbk + 1][:, :, 0:1])
                d = dpool.tile([128, 4 * NQ * 256], FP16, tag="d")
                d4 = d[:].rearrange("p (i q x) -> p i q x", i=4, q=NQ)
                E = c[:, :, 0:256]
                E1 = c[:, :, 1:257]
                O = c[:, :, 272:528]
                O1 = c[:, :, 271:527]
                v.tensor_tensor(d4[:, 1], E, O, ALU.add)        # d1 = x1+x2
                v.tensor_tensor(d4[:, 2], O, E, ALU.subtract)   # d2 = x2-x1
                v.tensor_tensor(d4[:, 0], O1, O, ALU.subtract)  # d0 = x0-x2
                v.tensor_tensor(d4[:, 3], E, E1, ALU.subtract)  # d3 = x1-x3
                dbufs[sbk] = d4

            hist_sb(cpool, 0)
            hist_sb(cpool, 1)
            dtrans(0)
            hist_sb(cpool, 2)
            dtrans(1)
            hist_sb(cpool, 3)
            for sbk in range(4, NSB):
                hist_sb(cpool, sbk)

            # ---- conv1 (Winograd) / conv2 / fc : pipelined over s-blocks ----
            with tc.tile_pool(name="up", bufs=2, space="PSUM") as up, \
                 tc.tile_pool(name="p2", bufs=2, space="PSUM") as pp2, \
                 tc.tile_pool(name="pf", bufs=2, space="PSUM") as pf, \
                 tc.tile_pool(name="sp", bufs=2) as spool, \
                 tc.tile_pool(name="ob", bufs=2) as ob:

                def conv1_sb(sbk):
                    d4 = dbufs[sbk]
                    for co in range(2):
                        u = up.tile([128, 4 * 256], F32, tag="u")
                        u3 = u[:].rearrange("p (i x) -> p i x", i=4)
                        for i in (1, 2, 0, 3):
                            for q in range(NQ):
                                nc.tensor.matmul(
                                    u3[:, i, :], g1t5[:, i, q, co, :],
                                    d4[:, i, q, :],
                                    start=(q == 0), stop=(q == NQ - 1))
                        # A^T combine: y0 = u0+u1+u2, y1 = u1-u2-u3
                        # (DVE ops may touch at most one PSUM operand, no PSUM
                        # out; copy u1/u2 out via ACT so 2 of 4 DVE ops are
                        # pure-SBUF and skip the PSUM access latency)
                        sc = spool.tile([128, 4 * 256], F32, tag="s")
                        s4 = sc[:].rearrange("p (i x) -> p i x", i=4)
                        nc.scalar.copy(s4[:, 0], u3[:, 1])                    # u1
                        nc.scalar.copy(s4[:, 1], u3[:, 2])                    # u2
                        v.tensor_tensor(s4[:, 2], s4[:, 0], s4[:, 1], ALU.add)
                        v.tensor_tensor(s4[:, 3], s4[:, 0], s4[:, 1], ALU.subtract)
                        v.tensor_tensor(s4[:, 2], s4[:, 2], u3[:, 0], ALU.add)   # y0
                        v.tensor_tensor(s4[:, 3], s4[:, 3], u3[:, 3], ALU.subtract)  # y1
                        xv = x23[sbk][:, co, 16:528].rearrange(
                            "p (x two) -> p two x", two=2)
                        nc.scalar.activation(xv[:, 0, :], s4[:, 2], RELU,
                                             bias=s_b1[:, co:co + 1])
                        nc.scalar.activation(xv[:, 1, :], s4[:, 3], RELU,
                                             bias=s_b1[:, co:co + 1])
                    if sbk % SB_PER_BATCH == 0:
                        v.memset(x23[sbk][:, :, 15:16], 0.0)
                    else:
                        nc.scalar.copy(x23[sbk][:, :, 15:16], x23[sbk - 1][:, :, 527:528])

                def conv2_sb(sbk):
                    if sbk % SB_PER_BATCH == SB_PER_BATCH - 1:
                        v.memset(x23[sbk][:, :, 528:529], 0.0)
                    else:
                        nc.scalar.copy(x23[sbk][:, :, 528:529], x23[sbk + 1][:, :, 16:17])
                    ps2 = pp2.tile([64, 512], F32, tag="ps2")
                    mms = [(1, 0), (0, 0), (0, 1), (1, 1), (2, 0), (2, 1)]
                    for i, (k, q) in enumerate(mms):
                        nc.tensor.matmul(
                            ps2[:], w2t4[:, k, q, :],
                            x23[sbk][:, q, 15 + k: 527 + k],
                            start=(i == 0), stop=(i == len(mms) - 1))
                    nc.scalar.activation(
                        s_x3[0:64, 512 * sbk: 512 * (sbk + 1)], ps2[:],
                        RELU, bias=s_b2[:, 0:1])

                out4 = d_out.ap().rearrange("(b t p) c -> b p t c", t=4, p=128)

                def fc_sb(sbk):
                    o = ob.tile([128, 4 * CHORD_SIZE], FP16, tag="o")
                    o3 = o[:].rearrange("p (t c) -> p t c", t=4)
                    for i, t in enumerate(range(4 * sbk, 4 * sbk + 4)):
                        psf = pf.tile([128, CHORD_SIZE], F32, tag="psf")
                        nc.tensor.matmul(psf[:], s_x3[:, 128 * t: 128 * (t + 1)],
                                         s_fcwb[:, 0:CHORD_SIZE], start=True, stop=True)
                        nc.scalar.copy(o3[:, i, :], psf[:])
                    nc.sync.dma_start(out4[sbk], o3)

                for sbk in range(NSB):
                    conv1_sb(sbk)
                    if sbk + 2 < NSB:
                        dtrans(sbk + 2)
                    if sbk >= 1:
                        conv2_sb(sbk - 1)
                    if sbk >= 2:
                        fc_sb(sbk - 2)
                conv2_sb(NSB - 1)
                for sbk in range(NSB - 2, NSB):
                    fc_sb(sbk)

    nc.compile()
    return nc


_NC = None


def _get_nc():
    global _NC
    if _NC is None:
        _NC = _build_program()
    return _NC


def _host_prep(chord_emb, note_emb, conv1_w, conv1_b, conv2_w, conv2_b, fc_w, fc_b):
    """Shared (replicated) constant tensors."""
    note_emb = np.asarray(note_emb, np.float32)
    chord_emb = np.asarray(chord_emb, np.float32)
    w1 = np.asarray(conv1_w, np.float32)
    E = np.zeros((NBINS, 512), np.float32)
    E[0:NOTE_SIZE, 256:512] = note_emb
    E[NOTE_SIZE:NOTE_SIZE + CHORD_SIZE, 0:256] = chord_emb
    F = np.einsum('vc,ock->kvo', E, w1)                      # [3, 1024, 256]
    G = np.stack([F[0], (F[0] + F[1] + F[2]) * 0.5,
                  (F[0] - F[1] + F[2]) * 0.5, F[2]])         # [4, 1024, 256]
    g1t = G.reshape(4, NQ, 128, 2, 128).transpose(2, 0, 1, 3, 4)  # [128,i,q,c,o]
    g1t = np.ascontiguousarray(g1t, np.float16).reshape(128, -1)

    w2 = np.asarray(conv2_w, np.float32).reshape(64, 2, 128, 3)
    w2t = np.ascontiguousarray(w2.transpose(2, 3, 1, 0), np.float16).reshape(128, -1)

    fcwb = np.zeros((65, 152), np.float16)
    fcwb[0:64, 0:CHORD_SIZE] = np.asarray(fc_w, np.float16)
    fcwb[64, 0:CHORD_SIZE] = np.asarray(fc_b, np.float16)

    b1t = np.ascontiguousarray(
        np.asarray(conv1_b, np.float32).reshape(2, 128).T)
    b2t = np.asarray(conv2_b, np.float32).reshape(64, 1)

    ones16 = np.ones((1, P), np.float16)
    return g1t, w2t, fcwb, b1t, b2t, ones16


# even/odd position permutation within each 512-pos s-block (tiles: E0 E1 O0 O1)
_PERM = (np.arange(0, B * S, 512)[:, None]
         + np.concatenate([np.arange(0, 512, 2), np.arange(1, 512, 2)])[None, :]
         ).reshape(-1)

_GE = (np.arange(16)[None, :] >= np.arange(16)[:, None])     # j' >= j
_LT = (np.arange(16)[None, :] < np.arange(16)[:, None])      # j' <  j


def _scatter_tables(note, chord):
    """Host-side ragged-index bookkeeping: dedup + count + 1/cnt scaling.

    note [B*S, 16] int64 (already E/O-permuted), chord [B*S] -> per-position
    scatter index/value rows [B*S, 18] (slots 0..15 notes, 16 chord, 17 pad).
    """
    nf = note.astype(np.int16)
    valid = np.cumprod(nf != 0, axis=1).astype(bool)          # prefix mask
    cnt = np.maximum(valid.sum(1), 1).astype(np.float32)
    eqv = (nf[:, :, None] == nf[:, None, :]) & valid[:, None, :]
    val = (eqv & _GE[None]).sum(2)                            # dups at j' >= j
    isdup = (eqv & _LT[None]).any(2)
    fv = valid & ~isdup                                       # first occurrence
    sidx = np.where(fv, nf, np.int16(-1))
    sval = np.where(fv, val / cnt[:, None], 0.0).astype(np.float16)
    sidx18 = np.full((nf.shape[0], 18), -1, np.int16)
    sval18 = np.zeros((nf.shape[0], 18), np.float16)
    sidx18[:, 0:16] = sidx
    sval18[:, 0:16] = sval
    sidx18[:, 16] = chord.astype(np.int16) + NOTE_SIZE
    sval18[:, 16] = 1.0
    return sidx18, sval18


def _make_in_maps(chord_emb, note_emb, conv1_w, conv1_b, conv2_w, conv2_b,
                  fc_w, fc_b, note, chord):
    g1t, w2t, fcwb, b1t, b2t, ones16 = _host_prep(
        chord_emb, note_emb, conv1_w, conv1_b, conv2_w, conv2_b, fc_w, fc_b)
    note = np.asarray(note).reshape(B * S, N)[_PERM]
    chord = np.asarray(chord).reshape(B * S)[_PERM]
    sidx18, sval18 = _scatter_tables(note, chord)
    in_maps = []
    for c in range(NCORES):
        si = sidx18[P * c: P * (c + 1)]
        sv = sval18[P * c: P * (c + 1)]
        sidx = np.ascontiguousarray(
            si.reshape(NT, 128, 18).transpose(1, 0, 2)).reshape(128, -1)
        sval = np.ascontiguousarray(
            sv.reshape(NT, 128, 18).transpose(1, 0, 2)).reshape(128, -1)
        in_maps.append({
            "sidx": sidx, "sval": sval, "g1t": g1t, "w2t": w2t,
            "fcwb": fcwb, "b1t": b1t, "b2t": b2t, "ones16": ones16,
        })
    return in_maps


def kernel(chord_emb, note_emb, conv1_w, conv1_b, conv2_w, conv2_b, fc_w, fc_b,
           note, chord):
    nc = _get_nc()
    in_maps = _make_in_maps(chord_emb, note_emb, conv1_w, conv1_b,
                            conv2_w, conv2_b, fc_w, fc_b, note, chord)
    res = run_bass_kernel_spmd(nc, in_maps, list(range(NCORES)))
    outs = [res.results[c]["out"].reshape(BLOC, S, CHORD_SIZE)
            for c in range(NCORES)]
    return np.concatenate(outs, axis=0).astype(np.float32)
